# revision 31
# baseline (speedup 1.0000x reference)
"""Trainium2 Bass kernel for KMeans assignment (argmin over centroid distances).

Problem: x [131072, 768] f32, centroids [768, 2000] f32
Output:  argmin_k ||x_n - c_k||^2  -> int32 [131072]

Math: argmin_k(||x||^2 - 2 x.c_k + ||c_k||^2) = argmax_k(x.c_k - 0.5||c_k||^2).
Data-parallel over 8 cores (16384 rows each).

Pipeline (mode "bf16", default):
  phase 1 "screen": x is cast to bf16 and pre-transposed to the PE weight
    layout ON HOST, so each 128-row tile is just: DMA in -> 24 matmuls
    (bias folded in as a 2-row hi/lo matmul) -> Act casts psum to fp16
    scores (centered so fp16 quantization stays small) -> DVE max/max_index
    (fp16, 2x rate) writing top-8 values+indices into resident SBUF tiles
    -> single DMA of all results at the end.
  phase 2 "exact": rows whose top-2 fp16 margin < TH are rescored exactly:
    host gathers them, decomposes to bf16 hi/lo, pre-transposes; device does
    3 matmul passes (xh.ch + xh.cl + xl.ch), adds fp32 bias on DVE, fp32
    argmax. Score error ~1e-3, negligible vs typical top-2 margins.

Mode "fp8": phase 1 uses fp8e4 (e4m3) with DoubleRow perf mode (2 k-tiles
per matmul), then the bf16 screen runs as a mid-stage on flagged rows, then
exact. Thresholds tuned per stage.
"""

import os
import sys

for _p in ("/opt/trn_rl_repo",):
    if _p not in sys.path and os.path.isdir(_p):
        sys.path.insert(0, _p)

from contextlib import ExitStack

import numpy as np
import ml_dtypes

import concourse.bass as bass
import concourse.tile as tile
from concourse import bacc, mybir
from concourse.bass_utils import run_bass_kernel_spmd

BF16 = np.dtype(ml_dtypes.bfloat16)
NP8 = np.dtype(ml_dtypes.float8_e4m3)

N, D, K = 131072, 768, 2000
NCORES = 8
NSH = N // NCORES  # 16384 rows per core
P = 128
DT = D // P  # 6 contraction tiles
NT = NSH // P  # 128 row tiles per core
KOFF = [0, 512, 1024, 1536]
KW = [512, 512, 512, 464]
NB = 4

F32 = mybir.dt.float32
F16 = mybir.dt.float16
BF = mybir.dt.bfloat16
FP8 = mybir.dt.float8e4
U16 = mybir.dt.uint16

# Score centering: argmax is invariant to a per-row constant, so shift
# scores toward 0 to keep fp16 quantization fine-grained. E[0.5||c||^2]=384.
CENTER = 384.0

# Margin thresholds. Measured on hw (micro.py sc_* tests): bf16 screen
# score error rms 0.065 / max 0.29; fp8 screen rms 1.03 / max 5.4.
TH_BF16 = float(os.environ.get("KMEANS_TH2", "0.22"))
TH_FP8 = float(os.environ.get("KMEANS_TH1", "5.5"))


def build_screen(n_rows: int, mm: str):
    """Screen program: top-8 (value,index) per row via mm-dtype matmul.

    mm: 'bf16' or 'fp8' (DoubleRow).
    Inputs:
      xt:  [P, nt*DT*P] mm-dtype, xt[p, t,d,n] = x[t*128+n, d*128+p]
      c:   bf16 [DT*P, K]  |  fp8 [3, P, 2*K] (DoubleRow pair layout)
      bias2: [2, K] bf16 hi/lo of (CENTER - 0.5||c_k||^2)
      ones:  [2, P] bf16
    Outputs:
      res_v: [P, nt*8] f32, res_i: [P, nt*8] u16

    Per tile: PE bias matmul (ones x bias2, start=True) + mm matmuls ->
    Act copies psum->sbuf f32 -> DVE max/max_index. PE is the bottleneck
    (~6us/tile); Act (~2.4us) and DVE (~4.5us) fit underneath.
    """
    assert n_rows % P == 0
    nt = n_rows // P
    nc = bacc.Bacc("TRN2", target_bir_lowering=False, debug=False)

    mdt = BF if mm == "bf16" else FP8
    xt_in = nc.dram_tensor("xt", [P, nt * DT * P], mdt, kind="ExternalInput").ap()
    if mm == "bf16":
        c_in = nc.dram_tensor("c", [DT * P, K], BF, kind="ExternalInput").ap()
    else:
        c_in = nc.dram_tensor("c", [DT // 2, P, 2 * K], FP8,
                              kind="ExternalInput").ap()
    bias2 = nc.dram_tensor("bias2", [2, K], BF, kind="ExternalInput").ap()
    ones = nc.dram_tensor("ones", [2, P], BF, kind="ExternalInput").ap()
    rv_out = nc.dram_tensor("res_v", [P, nt * 8], F32, kind="ExternalOutput").ap()
    ri_out = nc.dram_tensor("res_i", [P, nt * 8], U16, kind="ExternalOutput").ap()

    with tile.TileContext(nc) as tc, ExitStack() as ctx:
        const = ctx.enter_context(tc.tile_pool(name="const", bufs=1))
        xin_p = ctx.enter_context(tc.tile_pool(name="xin", bufs=4))
        sc_p = ctx.enter_context(tc.tile_pool(name="sc", bufs=1, space="PSUM"))
        ss_p = ctx.enter_context(tc.tile_pool(name="ss", bufs=2))
        res_p = ctx.enter_context(tc.tile_pool(name="res", bufs=1))

        # bias2/ones first (tiny, and every tile's first PE instruction needs
        # them), then centroids slice-by-slice in the exact order the PE
        # consumes them (d-outer, bank-inner) so the first row-tile's matmuls
        # start as soon as each slice lands instead of gating on the whole
        # 3MB load.
        bias_t = const.tile([2, K], BF, tag="bias2")
        nc.sync.dma_start(bias_t[:], bias2[:, :])
        ones_t = const.tile([2, P], BF, tag="ones")
        nc.sync.dma_start(ones_t[:], ones[:, :])
        c_tiles = []
        if mm == "bf16":
            c3 = c_in.rearrange("(t p) k -> t p k", p=P)
            for d in range(DT):
                ct = const.tile([P, K], BF, tag=f"c_{d}")
                c_tiles.append(ct)
            for d in range(DT):
                for b in range(NB):
                    nc.sync.dma_start(
                        c_tiles[d][:, KOFF[b]:KOFF[b] + KW[b]],
                        c3[d][:, KOFF[b]:KOFF[b] + KW[b]])
        else:
            for dp in range(DT // 2):
                ct = const.tile([P, 2, K], FP8, tag=f"c_{dp}")
                nc.sync.dma_start(
                    ct[:], c_in[dp].rearrange("p (two k) -> p two k", two=2))
                c_tiles.append(ct)

        res_v = res_p.tile([P, nt, 8], F32, tag="res_v")
        res_i = res_p.tile([P, nt, 8], U16, tag="res_i")

        xt3 = xt_in.rearrange("p (t f) -> p t f", t=nt)
        for t in range(nt):
            if mm == "bf16":
                xin = xin_p.tile([P, DT, P], mdt, name="xin")
            else:
                xin = xin_p.tile([P, DT // 2, 2, P], mdt, name="xin")
            nc.gpsimd.dma_start(xin[:], xt3[:, t, :])

            banks = [sc_p.tile([P, KW[b]], F32, tag=f"b{b}", name=f"bank{b}",
                               bufs=2) for b in range(NB)]
            # bias rows first so each bank's accumulation closes on the last d
            for b in range(NB):
                nc.tensor.matmul(
                    banks[b][:], ones_t[:], bias_t[:, KOFF[b]:KOFF[b] + KW[b]],
                    start=True, stop=False)
            if mm == "bf16":
                for d in range(DT):
                    for b in range(NB):
                        nc.tensor.matmul(
                            banks[b][:], xin[:, d, :],
                            c_tiles[d][:, KOFF[b]:KOFF[b] + KW[b]],
                            start=False, stop=(d == DT - 1))
            else:
                for dp in range(DT // 2):
                    for b in range(NB):
                        nc.tensor.matmul(
                            banks[b][:], xin[:, dp, :, :],
                            c_tiles[dp][:, :, KOFF[b]:KOFF[b] + KW[b]],
                            start=False, stop=(dp == DT // 2 - 1),
                            perf_mode=mybir.MatmulPerfMode.DoubleRow)

            ss = ss_p.tile([P, K], F32, name="ss", tag="ss")
            for b in range(NB):
                nc.scalar.copy(ss[:, KOFF[b]:KOFF[b] + KW[b]], banks[b][:])

            nc.vector.max(res_v[:, t, :], ss[:])
            nc.vector.max_index(res_i[:, t, :], res_v[:, t, :], ss[:])

            # stream results out in chunks so the final drain is short
            if (t + 1) % 32 == 0:
                lo, hi = t - 31, t + 1
                nc.sync.dma_start(rv_out[:, lo * 8:hi * 8],
                                  res_v[:, lo:hi, :])
                nc.sync.dma_start(ri_out[:, lo * 8:hi * 8],
                                  res_i[:, lo:hi, :])
        if nt % 32:
            lo = nt - (nt % 32)
            nc.sync.dma_start(rv_out[:, lo * 8:nt * 8], res_v[:, lo:nt, :])
            nc.sync.dma_start(ri_out[:, lo * 8:nt * 8], res_i[:, lo:nt, :])

    nc.compile()
    return nc


def build_exact(n_rows: int):
    """Exact rescore: bf16 hi/lo 3-pass matmul + fp32 bias + fp32 argmax.

    Inputs:
      xh, xl: [P, nt*DT*P] bf16 pre-transposed hi/lo of x rows
      ch, cl: [DT*P, K] bf16 hi/lo of centroids
      bias:   [P, K] f32 (CENTER - 0.5||c||^2, broadcast over partitions)
    Outputs:
      res_i: [P, nt*8] u16
    """
    assert n_rows % P == 0
    nt = n_rows // P
    nc = bacc.Bacc("TRN2", target_bir_lowering=False, debug=False)

    xh_in = nc.dram_tensor("xh", [P, nt * DT * P], BF, kind="ExternalInput").ap()
    xl_in = nc.dram_tensor("xl", [P, nt * DT * P], BF, kind="ExternalInput").ap()
    ch_in = nc.dram_tensor("ch", [DT * P, K], BF, kind="ExternalInput").ap()
    cl_in = nc.dram_tensor("cl", [DT * P, K], BF, kind="ExternalInput").ap()
    bias = nc.dram_tensor("bias", [P, K], F32, kind="ExternalInput").ap()
    ri_out = nc.dram_tensor("res_i", [P, nt * 8], U16, kind="ExternalOutput").ap()

    with tile.TileContext(nc) as tc, ExitStack() as ctx:
        const = ctx.enter_context(tc.tile_pool(name="const", bufs=1))
        xin_p = ctx.enter_context(tc.tile_pool(name="xin", bufs=3))
        sc_p = ctx.enter_context(tc.tile_pool(name="sc", bufs=2, space="PSUM"))
        ss_p = ctx.enter_context(tc.tile_pool(name="ss", bufs=2))
        res_p = ctx.enter_context(tc.tile_pool(name="res", bufs=1))
        mx_p = ctx.enter_context(tc.tile_pool(name="mx", bufs=2))

        # ch tiles first so the first tile's xh.ch matmuls can start before
        # cl finishes loading (whole tiles: fewer DGE setups wins at this
        # program size)
        c_tiles = {}
        ch3 = ch_in.rearrange("(t p) k -> t p k", p=P)
        cl3 = cl_in.rearrange("(t p) k -> t p k", p=P)
        for d in range(DT):
            ct = const.tile([P, K], BF, tag=f"c0_{d}", name=f"c0_{d}")
            nc.sync.dma_start(ct[:], ch3[d])
            c_tiles[(0, d)] = ct
        # bias on the Act queue (idle until the first psum copies) so it
        # neither delays the c stream nor the x tiles
        bias_t = const.tile([P, K], F32, tag="bias")
        nc.scalar.dma_start(bias_t[:], bias[:, :])
        for d in range(DT):
            ct = const.tile([P, K], BF, tag=f"c1_{d}", name=f"c1_{d}")
            nc.sync.dma_start(ct[:], cl3[d])
            c_tiles[(1, d)] = ct

        res_i = res_p.tile([P, nt, 8], U16, tag="res_i")

        xh3 = xh_in.rearrange("p (t f) -> p t f", t=nt)
        xl3 = xl_in.rearrange("p (t f) -> p t f", t=nt)
        for t in range(nt):
            # both x streams on gpsimd so they don't queue behind the 6MB of
            # centroid constants on the sync queue
            xh = xin_p.tile([P, DT, P], BF, name="xh", tag="xh")
            nc.gpsimd.dma_start(xh[:], xh3[:, t, :])
            xl = xin_p.tile([P, DT, P], BF, name="xl", tag="xl")
            nc.gpsimd.dma_start(xl[:], xl3[:, t, :])

            banks = [sc_p.tile([P, KW[b]], F32, tag=f"b{b}", name=f"bank{b}")
                     for b in range(NB)]
            # all xh.ch first (only needs ch resident), then the correction
            # passes xh.cl and xl.ch
            for d in range(DT):
                for b in range(NB):
                    nc.tensor.matmul(
                        banks[b][:], xh[:, d, :],
                        c_tiles[(0, d)][:, KOFF[b]:KOFF[b] + KW[b]],
                        start=(d == 0), stop=False)
            for d in range(DT):
                for b in range(NB):
                    nc.tensor.matmul(
                        banks[b][:], xh[:, d, :],
                        c_tiles[(1, d)][:, KOFF[b]:KOFF[b] + KW[b]],
                        start=False, stop=False)
            for d in range(DT):
                for b in range(NB):
                    nc.tensor.matmul(
                        banks[b][:], xl[:, d, :],
                        c_tiles[(0, d)][:, KOFF[b]:KOFF[b] + KW[b]],
                        start=False, stop=(d == DT - 1))

            ss = ss_p.tile([P, K], F32, name="ss")
            for b in range(NB):
                nc.vector.tensor_add(
                    ss[:, KOFF[b]:KOFF[b] + KW[b]], banks[b][:],
                    bias_t[:, KOFF[b]:KOFF[b] + KW[b]])

            mxv = mx_p.tile([P, 8], F32, tag="mxv", name="mxv")
            nc.vector.max(mxv[:], ss[:])
            nc.vector.max_index(res_i[:, t, :], mxv[:], ss[:])

        nc.sync.dma_start(ri_out[:, :], res_i[:])

    nc.compile()
    return nc


# ---------------- host-side prep ----------------

def _tpose(xs: np.ndarray) -> np.ndarray:
    """[rows, D] (any dtype) -> [P, nt*DT*P] with xt[p, t,d,n] = xs[t*P+n, d*P+p]."""
    nt = xs.shape[0] // P
    a = xs.reshape(nt, P, DT, P).transpose(3, 0, 2, 1)  # [p, t, d, n]
    return np.ascontiguousarray(a).reshape(P, nt * DT * P)


def _center_bias(centroids: np.ndarray) -> np.ndarray:
    c_norm = (centroids.astype(np.float64) ** 2).sum(axis=0)
    return (CENTER - 0.5 * c_norm).astype(np.float32)  # [K]


def prep_screen_const(centroids: np.ndarray, mm: str) -> dict:
    c = np.ascontiguousarray(centroids, dtype=np.float32)
    b = _center_bias(c)
    b_hi = b.astype(BF16)
    b_lo = (b - b_hi.astype(np.float32)).astype(BF16)
    out = {
        "bias2": np.ascontiguousarray(np.stack([b_hi, b_lo])),
        "ones": np.ones((2, P), dtype=BF16),
    }
    if mm == "bf16":
        out["c"] = c.astype(BF16)
    else:
        c8 = c.astype(NP8)  # [768, K]
        out["c"] = np.ascontiguousarray(
            c8.reshape(DT // 2, 2, P, K).transpose(0, 2, 1, 3)
        ).reshape(DT // 2, P, 2 * K)
    return out


def prep_exact_const(centroids: np.ndarray) -> dict:
    c = np.ascontiguousarray(centroids, dtype=np.float32)
    ch = c.astype(BF16)
    cl = (c - ch.astype(np.float32)).astype(BF16)
    bias = np.broadcast_to(_center_bias(c), (P, K)).copy()
    return {"ch": ch, "cl": cl, "bias": bias}


_NC_CACHE = {}
LAST_RESULTS = []  # (label, BassKernelResults) of most recent kernel() call


def _cached_nc(key, builder):
    if key not in _NC_CACHE:
        _NC_CACHE[key] = builder()
    return _NC_CACHE[key]


def _run_spmd(nc, in_maps, label):
    kw = {}
    if os.environ.get("KMEANS_TRACE"):
        kw["trace"] = True
        kw["tmpdir"] = os.environ.get("KMEANS_TRACE_DIR", "/tmp/km_trace") + "_" + label
        os.makedirs(kw["tmpdir"], exist_ok=True)
    res = run_bass_kernel_spmd(nc, in_maps, core_ids=list(range(NCORES)), **kw)
    LAST_RESULTS.append((label, res))
    return res


def _screen_once(x: np.ndarray, const: dict, mm: str, label: str):
    """Screen len(x) rows (must be NCORES*P multiple). Returns idx, margin."""
    rows = len(x)
    n_rows = rows // NCORES
    nc = _cached_nc(("screen", mm, n_rows), lambda: build_screen(n_rows, mm))
    np_dt = BF16 if mm == "bf16" else NP8
    xq = x.astype(np_dt)
    in_maps = []
    for i in range(NCORES):
        m = dict(const)
        m["xt"] = _tpose(xq[i * n_rows:(i + 1) * n_rows])
        in_maps.append(m)
    res = _run_spmd(nc, in_maps, label)
    nt = n_rows // P
    idxs, margins = [], []
    for i in range(NCORES):
        ri = res.results[i]["res_i"].reshape(P, nt, 8)
        rv = res.results[i]["res_v"].reshape(P, nt, 8).astype(np.float32)
        # row t*P+p  ->  ri[p, t]
        idxs.append(ri[:, :, 0].T.reshape(-1))
        margins.append((rv[:, :, 0] - rv[:, :, 1]).T.reshape(-1))
    return (np.concatenate(idxs).astype(np.int32), np.concatenate(margins))


def _run_screen_rows(x_rows: np.ndarray, const: dict, mm: str, label: str):
    """Screen an arbitrary-length row batch: pad to a bucketed per-core size
    (bounding compile count), chunk on overflow."""
    sizes = [512, 1024, 2048, 4096, 8192, 16384]
    need = (len(x_rows) + NCORES - 1) // NCORES
    per_core = min((s for s in sizes if s >= need), default=sizes[-1])
    cap = per_core * NCORES
    idx = np.empty(len(x_rows), dtype=np.int32)
    margin = np.empty(len(x_rows), dtype=np.float32)
    for s0 in range(0, len(x_rows), cap):
        chunk = x_rows[s0:s0 + cap]
        xp = np.zeros((cap, D), dtype=np.float32)
        xp[: len(chunk)] = chunk
        ci, cm = _screen_once(xp, const, mm, label)
        idx[s0:s0 + len(chunk)] = ci[: len(chunk)]
        margin[s0:s0 + len(chunk)] = cm[: len(chunk)]
    return idx, margin


def _run_exact(x_rows: np.ndarray, const: dict, label: str):
    """Exact top-1 for x_rows (padded internally). Returns idx [len(x_rows)]."""
    sizes = [256, 384, 512, 640, 768, 1024, 1536, 2048, 4096]
    need = (len(x_rows) + NCORES - 1) // NCORES
    per_core = min((s for s in sizes if s >= need), default=sizes[-1])
    out = np.empty(len(x_rows), dtype=np.int32)
    cap = per_core * NCORES
    for s0 in range(0, len(x_rows), cap):
        chunk = x_rows[s0:s0 + cap]
        total = cap
        xp = np.zeros((total, D), dtype=np.float32)
        xp[: len(chunk)] = chunk
        xh = xp.astype(BF16)
        xl = (xp - xh.astype(np.float32)).astype(BF16)
        nc = _cached_nc(("exact", per_core), lambda: build_exact(per_core))
        in_maps = []
        for i in range(NCORES):
            m = dict(const)
            m["xh"] = _tpose(xh[i * per_core:(i + 1) * per_core])
            m["xl"] = _tpose(xl[i * per_core:(i + 1) * per_core])
            in_maps.append(m)
        res = _run_spmd(nc, in_maps, label)
        nt = per_core // P
        parts = []
        for i in range(NCORES):
            ri = res.results[i]["res_i"].reshape(P, nt, 8)
            parts.append(ri[:, :, 0].T.reshape(-1))
        out[s0:s0 + len(chunk)] = np.concatenate(parts)[: len(chunk)]
    return out


def kernel(x: np.ndarray, centroids: np.ndarray) -> np.ndarray:
    mode = os.environ.get("KMEANS_MODE", "bf16")
    LAST_RESULTS.clear()
    x = np.ascontiguousarray(np.asarray(x), dtype=np.float32)
    centroids = np.ascontiguousarray(np.asarray(centroids), dtype=np.float32)

    exact_const = prep_exact_const(centroids)

    if mode == "fp8":
        const8 = prep_screen_const(centroids, "fp8")
        idx, margin = _screen_once(x, const8, "fp8", "p1fp8")
        flagged = np.flatnonzero(margin < TH_FP8)
        if len(flagged):
            constb = prep_screen_const(centroids, "bf16")
            idx2, margin2 = _run_screen_rows(
                x[flagged], constb, "bf16", "p2bf16")
            idx[flagged] = idx2
            flag2 = flagged[margin2 < TH_BF16]
            if len(flag2):
                idx[flag2] = _run_exact(x[flag2], exact_const, "p3exact")
        return idx

    # default: bf16 screen + exact rescore
    constb = prep_screen_const(centroids, "bf16")
    idx, margin = _screen_once(x, constb, "bf16", "p1bf16")
    flagged = np.flatnonzero(margin < TH_BF16)
    if os.environ.get("KMEANS_DEBUG"):
        print(f"[kmeans] flagged {len(flagged)}/{len(x)} "
              f"({100.0 * len(flagged) / len(x):.2f}%) at th={TH_BF16}",
              file=sys.stderr, flush=True)
    if len(flagged):
        idx[flagged] = _run_exact(x[flagged], exact_const, "p2exact")
    return idx


# revision 36
# speedup vs baseline: 1.0182x; 1.0182x over previous
"""Trainium2 Bass kernel for KMeans assignment (argmin over centroid distances).

Problem: x [131072, 768] f32, centroids [768, 2000] f32
Output:  argmin_k ||x_n - c_k||^2  -> int32 [131072]

Math: argmin_k(||x||^2 - 2 x.c_k + ||c_k||^2) = argmax_k(x.c_k - 0.5||c_k||^2).
Data-parallel over 8 cores (16384 rows each).

Pipeline (mode "bf16", default):
  phase 1 "screen": x is cast to bf16 and pre-transposed to the PE weight
    layout ON HOST, so each 128-row tile is just: DMA in -> 24 matmuls
    (bias folded in as a 2-row hi/lo matmul) -> Act casts psum to fp16
    scores (centered so fp16 quantization stays small) -> DVE max/max_index
    (fp16, 2x rate) writing top-8 values+indices into resident SBUF tiles
    -> single DMA of all results at the end.
  phase 2 "exact": rows whose top-2 fp16 margin < TH are rescored exactly:
    host gathers them, decomposes to bf16 hi/lo, pre-transposes; device does
    3 matmul passes (xh.ch + xh.cl + xl.ch), adds fp32 bias on DVE, fp32
    argmax. Score error ~1e-3, negligible vs typical top-2 margins.

Mode "fp8": phase 1 uses fp8e4 (e4m3) with DoubleRow perf mode (2 k-tiles
per matmul), then the bf16 screen runs as a mid-stage on flagged rows, then
exact. Thresholds tuned per stage.
"""

import os
import sys

for _p in ("/opt/trn_rl_repo",):
    if _p not in sys.path and os.path.isdir(_p):
        sys.path.insert(0, _p)

from contextlib import ExitStack

import numpy as np
import ml_dtypes

import concourse.bass as bass
import concourse.tile as tile
from concourse import bacc, mybir
from concourse.bass_utils import run_bass_kernel_spmd

BF16 = np.dtype(ml_dtypes.bfloat16)
NP8 = np.dtype(ml_dtypes.float8_e4m3)

N, D, K = 131072, 768, 2000
NCORES = 8
NSH = N // NCORES  # 16384 rows per core
P = 128
DT = D // P  # 6 contraction tiles
NT = NSH // P  # 128 row tiles per core
KOFF = [0, 512, 1024, 1536]
KW = [512, 512, 512, 464]
NB = 4

F32 = mybir.dt.float32
F16 = mybir.dt.float16
BF = mybir.dt.bfloat16
FP8 = mybir.dt.float8e4
U16 = mybir.dt.uint16

# Score centering: argmax is invariant to a per-row constant, so shift
# scores toward 0 to keep fp16 quantization fine-grained. E[0.5||c||^2]=384.
CENTER = 384.0

# Margin thresholds. Measured on hw (micro.py sc_* tests): bf16 screen
# score error rms 0.065 / max 0.29; fp8 screen rms 1.03 / max 5.4.
TH_BF16 = float(os.environ.get("KMEANS_TH2", "0.20"))
TH_FP8 = float(os.environ.get("KMEANS_TH1", "5.5"))


def build_screen(n_rows: int, mm: str):
    """Screen program: top-8 (value,index) per row via mm-dtype matmul.

    mm: 'bf16' or 'fp8' (DoubleRow).
    Inputs:
      xt:  [P, nt*DT*P] mm-dtype, xt[p, t,d,n] = x[t*128+n, d*128+p]
      c:   bf16 [DT*P, K]  |  fp8 [3, P, 2*K] (DoubleRow pair layout)
      bias2: [2, K] bf16 hi/lo of (CENTER - 0.5||c_k||^2)
      ones:  [2, P] bf16
    Outputs:
      res_v: [P, nt*8] f32, res_i: [P, nt*8] u16

    Per tile: PE bias matmul (ones x bias2, start=True) + mm matmuls ->
    Act copies psum->sbuf f32 -> DVE max/max_index. PE is the bottleneck
    (~6us/tile); Act (~2.4us) and DVE (~4.5us) fit underneath.
    """
    assert n_rows % P == 0
    nt = n_rows // P
    nc = bacc.Bacc("TRN2", target_bir_lowering=False, debug=False)

    mdt = BF if mm == "bf16" else FP8
    xt_in = nc.dram_tensor("xt", [P, nt * DT * P], mdt, kind="ExternalInput").ap()
    if mm == "bf16":
        c_in = nc.dram_tensor("c", [DT * P, K], BF, kind="ExternalInput").ap()
    else:
        c_in = nc.dram_tensor("c", [DT // 2, P, 2 * K], FP8,
                              kind="ExternalInput").ap()
    bias2 = nc.dram_tensor("bias2", [2, K], BF, kind="ExternalInput").ap()
    ones = nc.dram_tensor("ones", [2, P], BF, kind="ExternalInput").ap()
    rv_out = nc.dram_tensor("res_v", [P, nt * 8], F32, kind="ExternalOutput").ap()
    ri_out = nc.dram_tensor("res_i", [P, nt * 8], U16, kind="ExternalOutput").ap()

    with tile.TileContext(nc) as tc, ExitStack() as ctx:
        const = ctx.enter_context(tc.tile_pool(name="const", bufs=1))
        xin_p = ctx.enter_context(tc.tile_pool(name="xin", bufs=4))
        sc_p = ctx.enter_context(tc.tile_pool(name="sc", bufs=1, space="PSUM"))
        ss_p = ctx.enter_context(tc.tile_pool(name="ss", bufs=2))
        res_p = ctx.enter_context(tc.tile_pool(name="res", bufs=1))

        # bias2/ones first (tiny, and every tile's first PE instruction needs
        # them), then centroids slice-by-slice in the exact order the PE
        # consumes them (d-outer, bank-inner) so the first row-tile's matmuls
        # start as soon as each slice lands instead of gating on the whole
        # 3MB load.
        bias_t = const.tile([2, K], BF, tag="bias2")
        nc.sync.dma_start(bias_t[:], bias2[:, :])
        ones_t = const.tile([2, P], BF, tag="ones")
        nc.sync.dma_start(ones_t[:], ones[:, :])
        c_tiles = []
        if mm == "bf16":
            c3 = c_in.rearrange("(t p) k -> t p k", p=P)
            for d in range(DT):
                ct = const.tile([P, K], BF, tag=f"c_{d}")
                c_tiles.append(ct)
            for d in range(DT):
                for b in range(NB):
                    nc.sync.dma_start(
                        c_tiles[d][:, KOFF[b]:KOFF[b] + KW[b]],
                        c3[d][:, KOFF[b]:KOFF[b] + KW[b]])
        else:
            for dp in range(DT // 2):
                ct = const.tile([P, 2, K], FP8, tag=f"c_{dp}")
                nc.sync.dma_start(
                    ct[:], c_in[dp].rearrange("p (two k) -> p two k", two=2))
                c_tiles.append(ct)

        res_v = res_p.tile([P, nt, 8], F32, tag="res_v")
        res_i = res_p.tile([P, nt, 8], U16, tag="res_i")

        xt3 = xt_in.rearrange("p (t f) -> p t f", t=nt)
        for t in range(nt):
            if mm == "bf16":
                xin = xin_p.tile([P, DT, P], mdt, name="xin")
            else:
                xin = xin_p.tile([P, DT // 2, 2, P], mdt, name="xin")
            nc.gpsimd.dma_start(xin[:], xt3[:, t, :])

            banks = [sc_p.tile([P, KW[b]], F32, tag=f"b{b}", name=f"bank{b}",
                               bufs=2) for b in range(NB)]
            # bias rows first so each bank's accumulation closes on the last d
            for b in range(NB):
                nc.tensor.matmul(
                    banks[b][:], ones_t[:], bias_t[:, KOFF[b]:KOFF[b] + KW[b]],
                    start=True, stop=False)
            if mm == "bf16":
                for d in range(DT):
                    for b in range(NB):
                        nc.tensor.matmul(
                            banks[b][:], xin[:, d, :],
                            c_tiles[d][:, KOFF[b]:KOFF[b] + KW[b]],
                            start=False, stop=(d == DT - 1))
            else:
                for dp in range(DT // 2):
                    for b in range(NB):
                        nc.tensor.matmul(
                            banks[b][:], xin[:, dp, :, :],
                            c_tiles[dp][:, :, KOFF[b]:KOFF[b] + KW[b]],
                            start=False, stop=(dp == DT // 2 - 1),
                            perf_mode=mybir.MatmulPerfMode.DoubleRow)

            ss = ss_p.tile([P, K], F32, name="ss", tag="ss")
            for b in range(NB):
                nc.scalar.copy(ss[:, KOFF[b]:KOFF[b] + KW[b]], banks[b][:])

            nc.vector.max(res_v[:, t, :], ss[:])
            nc.vector.max_index(res_i[:, t, :], res_v[:, t, :], ss[:])

            # stream results out in chunks so the final drain is short
            if (t + 1) % 32 == 0:
                lo, hi = t - 31, t + 1
                nc.sync.dma_start(rv_out[:, lo * 8:hi * 8],
                                  res_v[:, lo:hi, :])
                nc.sync.dma_start(ri_out[:, lo * 8:hi * 8],
                                  res_i[:, lo:hi, :])
        if nt % 32:
            lo = nt - (nt % 32)
            nc.sync.dma_start(rv_out[:, lo * 8:nt * 8], res_v[:, lo:nt, :])
            nc.sync.dma_start(ri_out[:, lo * 8:nt * 8], res_i[:, lo:nt, :])

    nc.compile()
    return nc


def build_exact(n_rows: int):
    """Exact rescore: bf16 hi/lo 3-pass matmul + fp32 bias + fp32 argmax.

    Inputs:
      xh, xl: [P, nt*DT*P] bf16 pre-transposed hi/lo of x rows
      ch, cl: [DT*P, K] bf16 hi/lo of centroids
      bias:   [P, K] f32 (CENTER - 0.5||c||^2, broadcast over partitions)
    Outputs:
      res_i: [P, nt*8] u16
    """
    assert n_rows % P == 0
    nt = n_rows // P
    nc = bacc.Bacc("TRN2", target_bir_lowering=False, debug=False)

    xh_in = nc.dram_tensor("xh", [P, nt * DT * P], BF, kind="ExternalInput").ap()
    xl_in = nc.dram_tensor("xl", [P, nt * DT * P], BF, kind="ExternalInput").ap()
    ch_in = nc.dram_tensor("ch", [DT * P, K], BF, kind="ExternalInput").ap()
    cl_in = nc.dram_tensor("cl", [DT * P, K], BF, kind="ExternalInput").ap()
    bias = nc.dram_tensor("bias", [P, K], F32, kind="ExternalInput").ap()
    ri_out = nc.dram_tensor("res_i", [P, nt * 8], U16, kind="ExternalOutput").ap()

    with tile.TileContext(nc) as tc, ExitStack() as ctx:
        const = ctx.enter_context(tc.tile_pool(name="const", bufs=1))
        xin_p = ctx.enter_context(tc.tile_pool(name="xin", bufs=3))
        sc_p = ctx.enter_context(tc.tile_pool(name="sc", bufs=2, space="PSUM"))
        ss_p = ctx.enter_context(tc.tile_pool(name="ss", bufs=2))
        res_p = ctx.enter_context(tc.tile_pool(name="res", bufs=1))
        mx_p = ctx.enter_context(tc.tile_pool(name="mx", bufs=2))

        # const order matches PE/DVE need order: ch (passes 1+2), cl
        # (pass 3), bias (DVE add, after the first tile's matmuls)
        c_tiles = {}
        ch3 = ch_in.rearrange("(t p) k -> t p k", p=P)
        cl3 = cl_in.rearrange("(t p) k -> t p k", p=P)
        for d in range(DT):
            ct = const.tile([P, K], BF, tag=f"c0_{d}", name=f"c0_{d}")
            nc.sync.dma_start(ct[:], ch3[d])
            c_tiles[(0, d)] = ct
        for d in range(DT):
            ct = const.tile([P, K], BF, tag=f"c1_{d}", name=f"c1_{d}")
            nc.sync.dma_start(ct[:], cl3[d])
            c_tiles[(1, d)] = ct
        bias_t = const.tile([P, K], F32, tag="bias")
        nc.sync.dma_start(bias_t[:], bias[:, :])

        res_i = res_p.tile([P, nt, 8], U16, tag="res_i")

        xh3 = xh_in.rearrange("p (t f) -> p t f", t=nt)
        xl3 = xl_in.rearrange("p (t f) -> p t f", t=nt)
        for t in range(nt):
            # both x streams on gpsimd: the sync queue is busy with 6MB of
            # centroids at startup
            xh = xin_p.tile([P, DT, P], BF, name="xh", tag="xh")
            nc.gpsimd.dma_start(xh[:], xh3[:, t, :])
            xl = xin_p.tile([P, DT, P], BF, name="xl", tag="xl")
            nc.gpsimd.dma_start(xl[:], xl3[:, t, :])

            banks = [sc_p.tile([P, KW[b]], F32, tag=f"b{b}", name=f"bank{b}")
                     for b in range(NB)]
            # pass order chosen so passes 1+2 need only ch (resident first):
            # xh.ch, xl.ch, then xh.cl once cl has landed
            for d in range(DT):
                for b in range(NB):
                    nc.tensor.matmul(
                        banks[b][:], xh[:, d, :],
                        c_tiles[(0, d)][:, KOFF[b]:KOFF[b] + KW[b]],
                        start=(d == 0), stop=False)
            for d in range(DT):
                for b in range(NB):
                    nc.tensor.matmul(
                        banks[b][:], xl[:, d, :],
                        c_tiles[(0, d)][:, KOFF[b]:KOFF[b] + KW[b]],
                        start=False, stop=False)
            for d in range(DT):
                for b in range(NB):
                    nc.tensor.matmul(
                        banks[b][:], xh[:, d, :],
                        c_tiles[(1, d)][:, KOFF[b]:KOFF[b] + KW[b]],
                        start=False, stop=(d == DT - 1))

            ss = ss_p.tile([P, K], F32, name="ss")
            for b in range(NB):
                nc.vector.tensor_add(
                    ss[:, KOFF[b]:KOFF[b] + KW[b]], banks[b][:],
                    bias_t[:, KOFF[b]:KOFF[b] + KW[b]])

            mxv = mx_p.tile([P, 8], F32, tag="mxv", name="mxv")
            nc.vector.max(mxv[:], ss[:])
            nc.vector.max_index(res_i[:, t, :], mxv[:], ss[:])

        nc.sync.dma_start(ri_out[:, :], res_i[:])

    nc.compile()
    return nc


# ---------------- host-side prep ----------------

def _tpose(xs: np.ndarray) -> np.ndarray:
    """[rows, D] (any dtype) -> [P, nt*DT*P] with xt[p, t,d,n] = xs[t*P+n, d*P+p]."""
    nt = xs.shape[0] // P
    a = xs.reshape(nt, P, DT, P).transpose(3, 0, 2, 1)  # [p, t, d, n]
    return np.ascontiguousarray(a).reshape(P, nt * DT * P)


def _center_bias(centroids: np.ndarray) -> np.ndarray:
    c_norm = (centroids.astype(np.float64) ** 2).sum(axis=0)
    return (CENTER - 0.5 * c_norm).astype(np.float32)  # [K]


def prep_screen_const(centroids: np.ndarray, mm: str) -> dict:
    c = np.ascontiguousarray(centroids, dtype=np.float32)
    b = _center_bias(c)
    b_hi = b.astype(BF16)
    b_lo = (b - b_hi.astype(np.float32)).astype(BF16)
    out = {
        "bias2": np.ascontiguousarray(np.stack([b_hi, b_lo])),
        "ones": np.ones((2, P), dtype=BF16),
    }
    if mm == "bf16":
        out["c"] = c.astype(BF16)
    else:
        c8 = c.astype(NP8)  # [768, K]
        out["c"] = np.ascontiguousarray(
            c8.reshape(DT // 2, 2, P, K).transpose(0, 2, 1, 3)
        ).reshape(DT // 2, P, 2 * K)
    return out


def prep_exact_const(centroids: np.ndarray) -> dict:
    c = np.ascontiguousarray(centroids, dtype=np.float32)
    ch = c.astype(BF16)
    cl = (c - ch.astype(np.float32)).astype(BF16)
    bias = np.broadcast_to(_center_bias(c), (P, K)).copy()
    return {"ch": ch, "cl": cl, "bias": bias}


_NC_CACHE = {}
LAST_RESULTS = []  # (label, BassKernelResults) of most recent kernel() call


def _cached_nc(key, builder):
    if key not in _NC_CACHE:
        _NC_CACHE[key] = builder()
    return _NC_CACHE[key]


def _run_spmd(nc, in_maps, label):
    kw = {}
    if os.environ.get("KMEANS_TRACE"):
        kw["trace"] = True
        kw["tmpdir"] = os.environ.get("KMEANS_TRACE_DIR", "/tmp/km_trace") + "_" + label
        os.makedirs(kw["tmpdir"], exist_ok=True)
    res = run_bass_kernel_spmd(nc, in_maps, core_ids=list(range(NCORES)), **kw)
    LAST_RESULTS.append((label, res))
    return res


def _screen_once(x: np.ndarray, const: dict, mm: str, label: str):
    """Screen len(x) rows (must be NCORES*P multiple). Returns idx, margin."""
    rows = len(x)
    n_rows = rows // NCORES
    nc = _cached_nc(("screen", mm, n_rows), lambda: build_screen(n_rows, mm))
    np_dt = BF16 if mm == "bf16" else NP8
    xq = x.astype(np_dt)
    in_maps = []
    for i in range(NCORES):
        m = dict(const)
        m["xt"] = _tpose(xq[i * n_rows:(i + 1) * n_rows])
        in_maps.append(m)
    res = _run_spmd(nc, in_maps, label)
    nt = n_rows // P
    idxs, margins = [], []
    for i in range(NCORES):
        ri = res.results[i]["res_i"].reshape(P, nt, 8)
        rv = res.results[i]["res_v"].reshape(P, nt, 8).astype(np.float32)
        # row t*P+p  ->  ri[p, t]
        idxs.append(ri[:, :, 0].T.reshape(-1))
        margins.append((rv[:, :, 0] - rv[:, :, 1]).T.reshape(-1))
    return (np.concatenate(idxs).astype(np.int32), np.concatenate(margins))


def _run_screen_rows(x_rows: np.ndarray, const: dict, mm: str, label: str):
    """Screen an arbitrary-length row batch: pad to a bucketed per-core size
    (bounding compile count), chunk on overflow."""
    sizes = [512, 1024, 2048, 4096, 8192, 16384]
    need = (len(x_rows) + NCORES - 1) // NCORES
    per_core = min((s for s in sizes if s >= need), default=sizes[-1])
    cap = per_core * NCORES
    idx = np.empty(len(x_rows), dtype=np.int32)
    margin = np.empty(len(x_rows), dtype=np.float32)
    for s0 in range(0, len(x_rows), cap):
        chunk = x_rows[s0:s0 + cap]
        xp = np.zeros((cap, D), dtype=np.float32)
        xp[: len(chunk)] = chunk
        ci, cm = _screen_once(xp, const, mm, label)
        idx[s0:s0 + len(chunk)] = ci[: len(chunk)]
        margin[s0:s0 + len(chunk)] = cm[: len(chunk)]
    return idx, margin


def _run_exact(x_rows: np.ndarray, const: dict, label: str):
    """Exact top-1 for x_rows (padded internally). Returns idx [len(x_rows)]."""
    sizes = [256, 384, 512, 640, 768, 1024, 1536, 2048, 4096]
    need = (len(x_rows) + NCORES - 1) // NCORES
    per_core = min((s for s in sizes if s >= need), default=sizes[-1])
    out = np.empty(len(x_rows), dtype=np.int32)
    cap = per_core * NCORES
    for s0 in range(0, len(x_rows), cap):
        chunk = x_rows[s0:s0 + cap]
        total = cap
        xp = np.zeros((total, D), dtype=np.float32)
        xp[: len(chunk)] = chunk
        xh = xp.astype(BF16)
        xl = (xp - xh.astype(np.float32)).astype(BF16)
        nc = _cached_nc(("exact", per_core), lambda: build_exact(per_core))
        in_maps = []
        for i in range(NCORES):
            m = dict(const)
            m["xh"] = _tpose(xh[i * per_core:(i + 1) * per_core])
            m["xl"] = _tpose(xl[i * per_core:(i + 1) * per_core])
            in_maps.append(m)
        res = _run_spmd(nc, in_maps, label)
        nt = per_core // P
        parts = []
        for i in range(NCORES):
            ri = res.results[i]["res_i"].reshape(P, nt, 8)
            parts.append(ri[:, :, 0].T.reshape(-1))
        out[s0:s0 + len(chunk)] = np.concatenate(parts)[: len(chunk)]
    return out


def kernel(x: np.ndarray, centroids: np.ndarray) -> np.ndarray:
    mode = os.environ.get("KMEANS_MODE", "bf16")
    LAST_RESULTS.clear()
    x = np.ascontiguousarray(np.asarray(x), dtype=np.float32)
    centroids = np.ascontiguousarray(np.asarray(centroids), dtype=np.float32)

    exact_const = prep_exact_const(centroids)

    if mode == "fp8":
        const8 = prep_screen_const(centroids, "fp8")
        idx, margin = _screen_once(x, const8, "fp8", "p1fp8")
        flagged = np.flatnonzero(margin < TH_FP8)
        if len(flagged):
            constb = prep_screen_const(centroids, "bf16")
            idx2, margin2 = _run_screen_rows(
                x[flagged], constb, "bf16", "p2bf16")
            idx[flagged] = idx2
            flag2 = flagged[margin2 < TH_BF16]
            if len(flag2):
                idx[flag2] = _run_exact(x[flag2], exact_const, "p3exact")
        return idx

    # default: bf16 screen + exact rescore
    constb = prep_screen_const(centroids, "bf16")
    idx, margin = _screen_once(x, constb, "bf16", "p1bf16")
    flagged = np.flatnonzero(margin < TH_BF16)
    if os.environ.get("KMEANS_DEBUG"):
        print(f"[kmeans] flagged {len(flagged)}/{len(x)} "
              f"({100.0 * len(flagged) / len(x):.2f}%) at th={TH_BF16}",
              file=sys.stderr, flush=True)
    if len(flagged):
        idx[flagged] = _run_exact(x[flagged], exact_const, "p2exact")
    return idx
